# revision 30
# baseline (speedup 1.0000x reference)
"""Causal multi-head attention (B=4, T=2048, D=1024, H=16) on 8 Trainium2 cores.

Sharding (data + tensor parallel): core c handles batch b = c//2 and head-group
g = c%2 (8 of the 16 heads). Wq/Wk/Wv are column-sharded by head, Wp is
row-sharded; the two per-batch partial outputs are summed on the host (this
replaces the device all-reduce — the host-side sum is the unshard step).

Changes vs the 245us bf16 baseline (now ~215us):
  - Q/K/V projections run as fp8(e4m3) DoubleRow matmuls with hi/lo error
    compensation: host splits x and 32*W into fp8 hi + lo residual pairs;
    per k-tile pair (t0,t1) three DR matmuls contract (hi,hi), (lo_W,hi_x),
    (hi_W,lo_x) across the pair -> 0.75 cycles/row vs bf16, ~0.1% error.
    All DR operands are strided pairs: stride-0 broadcast moving operands
    silently requantize on hardware (verified) and must not be used
  - score matmuls are DR: K is stored as a device-split fp8 hi/lo pair
    (near-exact), Q as single fp8 duplicated into [128, 2, QB]; 0.5
    cycles/row. The 1/8 softmax scale and the 32^2 weight prescale fold
    into the exp's scale = 2^-13 (exact)
  - denominators come for free from a ones-column appended to V (acc width
    65); this removes the 1088 N=1 denominator matmuls and their PSUM bank
  - attention phase A per head-pair: scores + exp into an SBUF E-block;
    phase B uses the E chunks as the STATIONARY operand (bf16: E precision
    is the binding constraint; fp8 E busts the 2e-2 gate), region-major for
    PSUM accumulation; then a per-partition reciprocal + tensor_scalar
    normalize and PE transposes back to [d,q] for the bf16 output projection
  - causal mask multiplies run on the (otherwise idle) gpsimd engine
  - attention is ACT(exp)/PE-balanced; projections of the NEXT t-block, the
    deferred output projection of the PREVIOUS q-block, and the previous
    head-pair's phase B are interleaved as PE filler with deadline-scheduled
    pops emitted AFTER each k-tile-pair's score matmuls (a chain dispatcher
    keeps the pproj pool deadlock-free)
"""
import numpy as np

T = 2048
D = 1024
B = 4
H = 16
HL = 8            # heads per core
NP = 4            # head pairs per core
QB = 512          # q-block width
NQB = T // QB
NKT = T // 128
# host weight prescale for fp8 range; exp scale = 0.125/WS^2 (a power of 2,
# exact). NOTE device float8e4 is the IEEE e4m3 variant: max finite 240,
# values above ~248 become inf — WS=32 keeps |Q|,|K| < ~140.
WS = 32.0

_COMPILED = None


# --------------------------------------------------------------------------
# bass kernel build
# --------------------------------------------------------------------------
def _build_bass():
    import concourse.bass as bass
    import concourse.mybir as mybir
    from concourse.tile import TileContext

    F32 = mybir.dt.float32
    BF16 = mybir.dt.bfloat16
    FP8 = mybir.dt.float8e4
    Act = mybir.ActivationFunctionType
    Alu = mybir.AluOpType
    DR = mybir.MatmulPerfMode.DoubleRow
    ESCALE = 0.125 / (WS * WS)   # 2^-13, exact in f32

    nc = bass.Bass()
    # x pre-tiled on host: [p, tb, ktile, hi/lo, tok] flattened to 2D so
    # every SBUF load is a contiguous slice (DMA APs cap at 3 free dims)
    xt8 = nc.dram_tensor("xt8", [128, NQB * 8 * 2 * QB], FP8,
                         kind="ExternalInput")
    wq8 = nc.dram_tensor("wq8", [D, 2, 512], FP8, kind="ExternalInput")
    wk8 = nc.dram_tensor("wk8", [D, 2, 512], FP8, kind="ExternalInput")
    wv8 = nc.dram_tensor("wv8", [D, 2, 512], FP8, kind="ExternalInput")
    wp = nc.dram_tensor("wp", [512, D], BF16, kind="ExternalInput")
    mask1 = nc.dram_tensor("mask1", [128, 128], BF16, kind="ExternalInput")
    iden = nc.dram_tensor("iden", [128, 128], BF16, kind="ExternalInput")
    outt = nc.dram_tensor("outt", [D, T], BF16, kind="ExternalOutput")

    with TileContext(nc) as tc, nc.allow_low_precision(reason="fp8 pipeline"):
        with tc.tile_pool(name="wts", bufs=1) as wts, \
             tc.tile_pool(name="xp", bufs=2) as xp, \
             tc.tile_pool(name="big", bufs=1) as big, \
             tc.tile_pool(name="qtp", bufs=12) as qtp, \
             tc.tile_pool(name="ctp", bufs=12) as ctp, \
             tc.tile_pool(name="sm", bufs=1) as sm, \
             tc.tile_pool(name="osb", bufs=16) as osb, \
             tc.tile_pool(name="ebp", bufs=2) as ebp, \
             tc.tile_pool(name="cnp", bufs=2) as cnp, \
             tc.tile_pool(name="pproj", bufs=2, space="PSUM") as pproj, \
             tc.tile_pool(name="pst", bufs=2, space="PSUM") as pst, \
             tc.tile_pool(name="pca", bufs=2, space="PSUM") as pca:

            # ---------------- weights / constants ----------------
            wq_t = wts.tile([128, 8, 2, 512], FP8, tag="wq")
            wk_t = wts.tile([128, 8, 2, 512], FP8, tag="wk")
            wv_t = wts.tile([128, 8, 2, 512], FP8, tag="wv")
            wp_t = wts.tile([128, 4, 1024], BF16, tag="wp")
            wqr = wq8[:].rearrange("(n p) two m -> p n two m", p=128)
            wkr = wk8[:].rearrange("(n p) two m -> p n two m", p=128)
            wvr = wv8[:].rearrange("(n p) two m -> p n two m", p=128)
            wpr = wp[:].rearrange("(n p) m -> p n m", p=128)

            x_tiles = {}

            def load_x(tb, split=2):
                x_t = xp.tile([128, 8, 2, QB], FP8, tag="x", name=f"x_t{tb}")
                xr = xt8[:, 8 * 2 * QB * tb:8 * 2 * QB * (tb + 1)].rearrange(
                    "p (n two m) -> p n two m", n=8, two=2)
                step = 8 // split
                for kk in range(0, 8, step):
                    nc.sync.dma_start(x_t[:, kk:kk + step, :, :],
                                      xr[:, kk:kk + step, :, :])
                x_tiles[tb] = x_t

            # startup: small leading slices of wq/x0 so the first Q matmuls
            # start as early as possible, then the remainder in halves
            x_t0 = xp.tile([128, 8, 2, QB], FP8, tag="x", name="x_t0")
            xr0 = xt8[:, 0:8 * 2 * QB].rearrange(
                "p (n two m) -> p n two m", n=8, two=2)
            nc.sync.dma_start(wq_t[:, 0:2, :, :], wqr[:, 0:2, :, :])
            nc.scalar.dma_start(x_t0[:, 0:2, :, :], xr0[:, 0:2, :, :])
            nc.scalar.dma_start(wq_t[:, 2:5, :, :], wqr[:, 2:5, :, :])
            nc.sync.dma_start(x_t0[:, 2:5, :, :], xr0[:, 2:5, :, :])
            nc.sync.dma_start(wq_t[:, 5:8, :, :], wqr[:, 5:8, :, :])
            nc.sync.dma_start(x_t0[:, 5:8, :, :], xr0[:, 5:8, :, :])
            x_tiles[0] = x_t0
            nc.sync.dma_start(wk_t[:, 0:4, :, :], wkr[:, 0:4, :, :])
            nc.sync.dma_start(wk_t[:, 4:8, :, :], wkr[:, 4:8, :, :])
            nc.sync.dma_start(wv_t[:, 0:4, :, :], wvr[:, 0:4, :, :])
            nc.sync.dma_start(wv_t[:, 4:8, :, :], wvr[:, 4:8, :, :])
            m1 = sm.tile([128, 128], BF16, tag="m1")
            nc.sync.dma_start(m1[:], mask1[:])
            idt = sm.tile([128, 128], BF16, tag="iden")
            nc.sync.dma_start(idt[:], iden[:])
            for kk in range(4):
                nc.sync.dma_start(wp_t[:, kk, :], wpr[:, kk, :])

            # K as fp8 hi/lo pairs: [part(2 heads), pair, ktile, hi/lo, keys]
            khl = big.tile([128, NP, NKT, 2, 128], FP8, tag="khl")
            # V with a ones column (65th) for fused denominators
            va_t = big.tile([128, NKT, HL, 65], BF16, tag="va")
            for i in range(NKT):
                nc.gpsimd.memset(va_t[:, i, :, 64:65], 1.0)

            # ---------------- filler machinery ----------------
            # filler work is organized as CHAINS (one PSUM accumulation each,
            # 4 units of ~3 matmuls); a central dispatcher guarantees at
            # most one chain is mid-flight so the pproj pool (2 bufs) always
            # has a slot for the transpose matmul.
            # due_q/due_kv[tb] hold chains of t-block tb with a deadline
            # inside attention(tb); free_chains can run anywhere.
            free_chains = []
            due_q = {}
            due_kv = {}
            active = []

            def emit_unit(select):
                if not active:
                    ch = select()
                    if ch is None:
                        return False
                    active.extend(ch)
                active.pop(0)()
                return True

            def finish_active():
                while active:
                    active.pop(0)()

            def pop_free(n=1, tb_next=None):
                # may eat due_kv[tb_next] except for the final block, whose
                # K/V chains are the only filler reserve left at that point
                def sel():
                    if free_chains:
                        return free_chains.pop(0)[1]
                    if tb_next is not None and due_q.get(tb_next):
                        return due_q[tb_next].pop(0)
                    if tb_next is not None and tb_next < NQB - 1:
                        d = due_kv.get(tb_next)
                        if d:
                            if d["v"]:
                                return d["v"].pop(0)
                            for pp in (1, 2, 3):
                                if d["k"].get(pp):
                                    return d["k"][pp].pop(0)
                    return None

                for _ in range(n):
                    if not emit_unit(sel):
                        break

            def drain_expired(j):
                # oproj chains past expiry must be done before ctx(j) tiles
                # reuse their pool slots (scan all: holdback breaks FIFO)
                finish_active()
                rest = []
                for exp, ch in free_chains:
                    if exp <= j:
                        for u in ch:
                            u()
                    else:
                        rest.append((exp, ch))
                free_chains[:] = rest

            def drain_chains(chains):
                finish_active()
                while chains:
                    for u in chains.pop(0):
                        u()

            def drain_kv(tb):
                d = due_kv.get(tb)
                if d:
                    drain_chains(d["v"])
                    for p in (1, 2, 3):
                        drain_chains(d["k"][p])

            qt_tiles = {}
            ctx_tiles = {}

            def make_proj_chain(kind, tb, p):
                """One projection chain: 4 units, each 3 DR matmuls covering
                a k-tile pair via the hi/lo 3-term scheme, plus final copies.
                kind: 'q' -> fp8 qt tile, 'k' -> khl hi/lo pair,
                'v' -> va_t tiles.
                """
                st = {"ps": None}
                x_t = x_tiles[tb]

                def unit(tp, kind=kind, tb=tb, p=p):
                    def f():
                        if tp == 0:
                            st["ps"] = pproj.tile(
                                [128, QB], F32, tag="mm",
                                name=f"ps_{kind}{tb}_{p}")
                        ps = st["ps"]
                        t0 = 2 * tp
                        tt = slice(t0, t0 + 2)
                        cs = slice(128 * p, 128 * (p + 1))
                        # 3-term hi/lo over a k-tile pair, all strided-pair
                        # APs (stride-0 broadcast DR is broken on hardware):
                        # hi*hi for both k-tiles, lo(stationary)*hi, hi*lo
                        if kind == "v":
                            # stationary = x chunk (tokens), moving = Wv
                            nc.tensor.matmul(
                                ps[:], x_t[:, tt, 0, cs], wv_t[:, tt, 0, :],
                                start=(tp == 0), stop=False, perf_mode=DR)
                            nc.tensor.matmul(
                                ps[:], x_t[:, tt, 1, cs], wv_t[:, tt, 0, :],
                                start=False, stop=False, perf_mode=DR)
                            nc.tensor.matmul(
                                ps[:], x_t[:, tt, 0, cs], wv_t[:, tt, 1, :],
                                start=False, stop=(tp == 3), perf_mode=DR)
                        else:
                            w_t = wq_t if kind == "q" else wk_t
                            nc.tensor.matmul(
                                ps[:], w_t[:, tt, 0, cs], x_t[:, tt, 0, :],
                                start=(tp == 0), stop=False, perf_mode=DR)
                            nc.tensor.matmul(
                                ps[:], w_t[:, tt, 1, cs], x_t[:, tt, 0, :],
                                start=False, stop=False, perf_mode=DR)
                            nc.tensor.matmul(
                                ps[:], w_t[:, tt, 0, cs], x_t[:, tt, 1, :],
                                start=False, stop=(tp == 3), perf_mode=DR)
                        if tp == 3:
                            if kind == "q":
                                # duplicated halves: DR moving operand for
                                # the score matmuls without stride-0
                                q_tile = qtp.tile([128, 2, QB], FP8,
                                                  tag="qt",
                                                  name=f"qt{tb}_{p}")
                                nc.vector.tensor_copy(q_tile[:, 0, :], ps[:])
                                nc.vector.tensor_copy(q_tile[:, 1, :], ps[:])
                                qt_tiles[(tb, p)] = q_tile
                            elif kind == "k":
                                kv = khl[:, p, 4 * tb:4 * (tb + 1), :, :]
                                psv = ps[:].rearrange(
                                    "p (c m) -> p c m", c=4)
                                nc.vector.tensor_copy(kv[:, :, 0, :], psv)
                                nc.vector.tensor_tensor(
                                    kv[:, :, 1, :], psv, kv[:, :, 0, :],
                                    op=Alu.subtract)
                            else:
                                nc.vector.tensor_copy(
                                    va_t[:, 4 * tb + p, :, 0:64],
                                    ps[:].rearrange("p (h d) -> p h d",
                                                    h=HL))
                    return f

                return [unit(tp) for tp in range(4)]

            def queue_proj(tb):
                due_q[tb] = [make_proj_chain("q", tb, p) for p in range(NP)]
                # v-class: all V chains + K(p=0) — needed by head-pair 0's
                # diagonal; k-class[p]: K(p) — needed by head-pair p's diagonal
                vs = [make_proj_chain("v", tb, p) for p in range(NP)]
                due_kv[tb] = {
                    "v": vs,
                    "k": {p: [make_proj_chain("k", tb, p)]
                          for p in (0, 1, 2, 3)},
                }

            def make_oproj(j):
                """Output projection for q-block j: 8 m-chunks x 4 p-acc."""
                units = []

                def unit(m, phalf, j=j):
                    st_key = ("pf", j, m)

                    def f():
                        if phalf == 0:
                            pf = pproj.tile([128, QB], F32, tag="mm",
                                            name=f"pf{j}_{m}")
                            _oproj_ps[st_key] = pf
                        pf = _oproj_ps[st_key]
                        for p in (phalf * 2, phalf * 2 + 1):
                            nc.tensor.matmul(
                                pf[:], wp_t[:, p, 128 * m:128 * (m + 1)],
                                ctx_tiles[(j, p)][:],
                                start=(p == 0), stop=(p == 3))
                        if phalf == 1:
                            ob = osb.tile([128, QB], BF16, tag="ob",
                                          name=f"ob{j}_{m}")
                            nc.vector.tensor_copy(ob[:], pf[:])
                            nc.sync.dma_start(
                                outt[128 * m:128 * (m + 1),
                                     QB * j:QB * (j + 1)], ob[:])
                    return f

                for m in range(8):
                    units.append([unit(m, 0), unit(m, 1)])
                return units

            _oproj_ps = {}


            # ---------------- attention ----------------
            # phase A per head-pair: DR scores + exp into an SBUF E-block.
            # phase B (E-stationary, region-major so each PSUM accumulation
            # group is consecutive): ctx[q,d]+den via the ones column, then
            # per-partition normalize and PE transposes back to [d,q].
            # phase B of pair p is queued and popped into pair p+1's stream.
            pending_pb = []

            def pop_pb(n=1):
                for _ in range(n):
                    if pending_pb:
                        pending_pb.pop(0)()

            def flush_pb():
                while pending_pb:
                    pending_pb.pop(0)()

            def emit_attention(j, q_per_pair=False):
                nkt_j = 4 * j + 4
                npairs = nkt_j // 2
                # previous block's last phase B must land before any filler
                # pop that might read its ctx tiles (deferred oproj)
                flush_pb()
                drain_expired(j)

                def pace_k0(ip):
                    # K(p0) must fully land before p0's diagonal S-mms
                    d = due_kv.get(j)
                    if not (d and d["k"].get(0)):
                        return
                    units_left = len(active) + sum(len(c) for c in d["k"][0])
                    pts_left = max(1, 2 * j - ip)
                    n = (units_left + pts_left - 1) // pts_left
                    k0 = d["k"][0]
                    pop = lambda: k0.pop(0) if k0 else None
                    for _ in range(n):
                        if not emit_unit(pop):
                            break
                    if not k0:
                        del d["k"][0]

                def pop_point(p, ip):
                    d = due_kv.get(j)
                    if d and (d["v"] or (active and p == 0)):
                        if p > 0:
                            drain_chains(d["v"])  # deadline passed
                        else:
                            # V feeds phase B (popped one pair later), so it
                            # only needs to land by the end of head-pair 0
                            units_left = len(active) + sum(
                                len(c) for c in d["v"])
                            pts_left = max(1, npairs - ip)
                            n = (units_left + pts_left - 1) // pts_left
                            pop = lambda: d["v"].pop(0) if d["v"] else None
                            for _ in range(n):
                                if not emit_unit(pop):
                                    break
                        return
                    if d and p < 3 and d["k"].get(p + 1):
                        kl = d["k"][p + 1]
                        emit_unit(lambda: kl.pop(0) if kl else None)
                        return
                    pop_free(1, tb_next=j + 1)

                for p in range(NP):
                    dq = due_q.get(j)
                    if q_per_pair and dq:
                        drain_chains([dq.pop(0)])  # q(p) at pair-p start
                    d = due_kv.get(j)
                    if d and d["k"].get(p):
                        drain_chains(d["k"][p])  # K(p) before p's diagonal
                    q_tile = qt_tiles[(j, p)]
                    eb = ebp.tile([128, 2, NKT, QB], BF16, tag="eb",
                                  name=f"eb{j}_{p}")
                    # ---- phase A ----
                    for ip in range(npairs):  # ascending k-tiles, s inner
                        if p == 0:
                            pace_k0(ip)
                        i0, i1 = 2 * ip, 2 * ip + 1
                        o0, o1 = i0 - 4 * j, i1 - 4 * j
                        cs0 = 0 if o0 < 0 else 128 * o0
                        for s in range(2):
                            hs = slice(64 * s, 64 * s + 64)
                            tp = (64 * s, 0)
                            st2 = pst.tile([128, 2, QB], F32, tag="st",
                                           name=f"st{j}_{p}_{s}_{ip}")
                            # i1's S-matmul widens to cs0: the extra 128-col
                            # chunk (q < k everywhere) feeds only phase-B
                            # regions that are skipped, and it makes the
                            # diagonal pair's exp one rectangular op
                            nc.tensor.matmul(
                                st2[:, 0, cs0:QB],
                                khl[hs, p, i0, :, :],
                                q_tile[hs, :, cs0:QB],
                                start=True, stop=True, tile_position=tp,
                                perf_mode=DR)
                            nc.tensor.matmul(
                                st2[:, 1, cs0:QB],
                                khl[hs, p, i1, :, :],
                                q_tile[hs, :, cs0:QB],
                                start=True, stop=True, tile_position=tp,
                                perf_mode=DR)
                            if o1 < 0:
                                nc.scalar.activation(
                                    eb[:, s, i0:i0 + 2, :], st2[:], Act.Exp,
                                    scale=ESCALE)
                            else:
                                nc.scalar.activation(
                                    eb[:, s, i0:i0 + 2, cs0:QB],
                                    st2[:, :, cs0:QB], Act.Exp, scale=ESCALE)
                                for (oo, ii) in ((o0, i0), (o1, i1)):
                                    if oo < 0:
                                        continue
                                    z = slice(128 * oo, 128 * (oo + 1))
                                    nc.gpsimd.tensor_tensor(
                                        eb[:, s, ii, z], eb[:, s, ii, z],
                                        m1[:], op=Alu.mult)
                        pop_point(p, ip)
                        pop_pb(2 if j == NQB - 1 else 1)
                    # ---- queue phase B ----
                    flush_pb()  # pair p-1 leftovers; frees acc slots
                    holders = {}

                    def pb_region(s, qc, p=p, j=j, eb=eb, holders=holders):
                        def f():
                            if ("acc", s) not in holders:
                                holders[("acc", s)] = pca.tile(
                                    [128, 4, 65], F32, tag="acc",
                                    name=f"acc{j}_{p}_{s}")
                            acc = holders[("acc", s)]
                            h = 2 * p + s
                            last_i = 4 * j + qc
                            for i in range(last_i + 1):
                                nc.tensor.matmul(
                                    acc[:, qc, :],
                                    eb[:, s, i, 128 * qc:128 * (qc + 1)],
                                    va_t[:, i, h, :],
                                    start=(i == 0), stop=(i == last_i))
                        return f

                    def pb_norm(p=p, j=j, holders=holders):
                        def f():
                            recip_t = sm.tile([128, 2, 4], F32, tag="recip",
                                              bufs=2, name=f"recip{j}_{p}")
                            ctx_n = cnp.tile([128, 2, 4, 64], BF16, tag="cn",
                                             name=f"cn{j}_{p}")
                            holders["cn"] = ctx_n
                            for s in range(2):
                                acc = holders[("acc", s)]
                                nc.vector.reciprocal(recip_t[:, s, :],
                                                     acc[:, :, 64])
                                for qc in range(4):
                                    nc.vector.tensor_scalar_mul(
                                        ctx_n[:, s, qc, :],
                                        acc[:, qc, 0:64],
                                        recip_t[:, s, qc:qc + 1])
                        return f

                    def pb_transp(p=p, j=j, holders=holders):
                        def f():
                            ctx_n = holders["cn"]
                            tpp = pproj.tile([128, 2 * QB], BF16, tag="mm",
                                             name=f"tpp{j}_{p}")
                            for s in range(2):
                                for qc in range(4):
                                    nc.tensor.transpose(
                                        tpp[64 * s:64 * s + 64,
                                            128 * qc:128 * (qc + 1)],
                                        ctx_n[:, s, qc, :], idt[:],
                                        tile_position=(0, 64 * s))
                            ctxT = ctp.tile([128, QB], BF16, tag="ctx",
                                            name=f"ctx{j}_{p}")
                            nc.vector.tensor_copy(ctxT[:], tpp[:, 0:QB])
                            ctx_tiles[(j, p)] = ctxT
                        return f

                    for s2 in range(2):
                        for qc2 in range(4):
                            pending_pb.append(pb_region(s2, qc2))
                    pending_pb.append(pb_norm())
                    pending_pb.append(pb_transp())

            # ---------------- schedule ----------------
            load_x(1)
            # tb=0: attention(0) starts as soon as q(p0)+K(p0) land; the
            # remaining tb0 chains drain at pair starts / pop points
            queue_proj(0)
            drain_chains(due_q[0])
            drain_kv(0)
            queue_proj(1)
            emit_attention(0)

            for tb in range(1, NQB):
                drain_chains(due_q[tb])  # q tiles needed at head-pair starts
                if tb + 1 < NQB:
                    load_x(tb + 1)
                    queue_proj(tb + 1)
                free_chains.extend((tb + 1, c) for c in make_oproj(tb - 1))
                emit_attention(tb)

            # tail: leftover fillers, then the last pair's phase B
            # interleaved with partial (p0..p2) output chains; p3 finishes
            # after the final transpose lands
            finish_active()
            while free_chains:
                for u in free_chains.pop(0)[1]:
                    u()
            jf = NQB - 1
            ob_part = wts.tile([128, 8, QB], BF16, tag="obp")

            def p1_unit(m):
                def f():
                    pf = pproj.tile([128, QB], F32, tag="mm",
                                    name=f"pp1_{m}")
                    for p in (0, 1, 2):
                        nc.tensor.matmul(
                            pf[:], wp_t[:, p, 128 * m:128 * (m + 1)],
                            ctx_tiles[(jf, p)][:],
                            start=(p == 0), stop=(p == 2))
                    nc.vector.tensor_copy(ob_part[:, m, :], pf[:])
                return f

            pass1 = [p1_unit(m) for m in range(8)]
            while pending_pb or pass1:
                if pending_pb:
                    pending_pb.pop(0)()
                if pass1:
                    pass1.pop(0)()
            for m in range(8):
                pf2 = pproj.tile([128, QB], F32, tag="mm", name=f"pf2_{m}")
                nc.tensor.matmul(pf2[:], wp_t[:, 3, 128 * m:128 * (m + 1)],
                                 ctx_tiles[(jf, 3)][:],
                                 start=True, stop=True)
                ob = osb.tile([128, QB], BF16, tag="ob", name=f"obf{m}")
                nc.vector.tensor_tensor(ob[:], ob_part[:, m, :], pf2[:],
                                        op=Alu.add)
                nc.sync.dma_start(
                    outt[128 * m:128 * (m + 1), QB * jf:T], ob[:])
    return nc


def _split_waits(nc, limit=1):
    """This walrus build accepts only one sync wait per TPB_CTRL instruction;
    move excess waits onto preceding same-engine NOPs."""
    import concourse.mybir as mybir
    for f in nc.m.functions:
        for bb in f.blocks:
            new_insts = []
            for inst in bb.instructions:
                si = inst.sync_info
                if si is not None and si.on_wait and len(si.on_wait) > limit:
                    waits = list(si.on_wait)
                    k = 0
                    while len(waits) - k > limit:
                        chunk = waits[k:k + limit]
                        k += limit
                        nop = mybir.InstNoOp(name=f"{inst.name}_ws{k}")
                        nop.engine = inst.engine
                        nop.sync_info = mybir.SyncInfo(on_wait=chunk, on_update=[])
                        new_insts.append(nop)
                    si.on_wait = waits[k:]
                new_insts.append(inst)
            bb.instructions = new_insts


# --------------------------------------------------------------------------
# compile + SPMD execution via PJRT (axon) — jit once, reuse
# --------------------------------------------------------------------------
class _Compiled:
    def __init__(self, n_cores=8):
        import jax
        from jax.sharding import Mesh, PartitionSpec
        from jax.experimental.shard_map import shard_map
        import concourse.mybir as mybir
        from concourse.bass2jax import (_bass_exec_p, install_neuronx_cc_hook,
                                        partition_id_tensor)

        nc = _build_bass()
        _split_waits(nc)
        install_neuronx_cc_hook()
        partition_name = nc.partition_id_tensor.name if nc.partition_id_tensor else None
        in_names, out_names, out_avals, zero_outs = [], [], [], []
        for alloc in nc.m.functions[0].allocations:
            if not isinstance(alloc, mybir.MemoryLocationSet):
                continue
            name = alloc.memorylocations[0].name
            if alloc.kind == "ExternalInput":
                if name != partition_name:
                    in_names.append(name)
            elif alloc.kind == "ExternalOutput":
                shape = tuple(alloc.tensor_shape)
                dtype = mybir.dt.np(alloc.dtype)
                out_names.append(name)
                out_avals.append(jax.core.ShapedArray(shape, dtype))
                zero_outs.append(np.zeros(shape, dtype))
        n_params = len(in_names)
        all_in_names = list(in_names) + list(out_names)
        if partition_name is not None:
            all_in_names.append(partition_name)

        def _body(*args):
            operands = list(args)
            if partition_name is not None:
                operands.append(partition_id_tensor())
            outs = _bass_exec_p.bind(
                *operands,
                out_avals=tuple(out_avals),
                in_names=tuple(all_in_names),
                out_names=tuple(out_names),
                lowering_input_output_aliases=(),
                sim_require_finite=True,
                sim_require_nnan=True,
                nc=nc,
            )
            return tuple(outs)

        devices = jax.devices()[:n_cores]
        assert len(devices) >= n_cores, f"need {n_cores} cores, have {len(devices)}"
        self.n_cores = n_cores
        self.in_names, self.out_names = in_names, out_names
        self.out_avals, self.zero_outs = out_avals, zero_outs
        mesh = Mesh(np.asarray(devices[:n_cores]), ("core",))
        in_specs = (PartitionSpec("core"),) * (n_params + len(out_names))
        out_specs = (PartitionSpec("core"),) * len(out_names)
        self.fn = jax.jit(
            shard_map(_body, mesh=mesh, in_specs=in_specs,
                      out_specs=out_specs, check_rep=False),
            keep_unused=True)

    def run(self, in_maps):
        import jax
        args = []
        for name in self.in_names:
            args.append(np.concatenate([np.asarray(m[name]) for m in in_maps], axis=0))
        for z in self.zero_outs:
            args.append(np.zeros((self.n_cores * z.shape[0], *z.shape[1:]), z.dtype))
        outs = self.fn(*args)
        jax.block_until_ready(outs)
        res = []
        for c in range(self.n_cores):
            d = {}
            for i, name in enumerate(self.out_names):
                a = np.asarray(outs[i]).reshape(self.n_cores, *self.out_avals[i].shape)[c]
                d[name] = a
            res.append(d)
        return res


# --------------------------------------------------------------------------
# host-side shard / unshard
# --------------------------------------------------------------------------
def _make_core_inputs(x, Wq, Wk, Wv, Wp, core):
    import ml_dtypes
    bf16 = ml_dtypes.bfloat16
    f8 = ml_dtypes.float8_e4m3
    g = core % 2
    b = core // 2
    rows = slice(512 * g, 512 * (g + 1))
    kl = np.arange(128)

    def hilo(a):
        """[N, M] f32 -> [N, 2, M] fp8 hi/lo pair"""
        hi = a.astype(f8)
        lo = (a - hi.astype(np.float32)).astype(f8)
        return np.ascontiguousarray(np.stack([hi, lo], axis=1))

    xt = x[b].T.astype(np.float32)
    # pre-tiled x: [p, tb, ktile, hi/lo, tok] -> [128, NQB*8*2*QB]
    xhl = hilo(xt)                                    # [1024, 2, 2048]
    xhl = xhl.reshape(8, 128, 2, NQB, QB)             # (n, p, two, tb, m)
    xhl = xhl.transpose(1, 3, 0, 2, 4)                # (p, tb, n, two, m)
    xhl = np.ascontiguousarray(xhl.reshape(128, NQB * 8 * 2 * QB))
    return {
        "xt8": xhl,
        "wq8": hilo(Wq[rows, :].T * WS),
        "wk8": hilo(Wk[rows, :].T * WS),
        "wv8": hilo(Wv[rows, :].T * WS),
        # V carries the WS scale; divide Wp by WS to undo it
        "wp": np.ascontiguousarray(Wp[:, rows].T / WS).astype(bf16),
        "mask1": (kl[:, None] <= kl[None, :]).astype(bf16),
        "iden": np.eye(128, dtype=np.float32).astype(bf16),
    }


def kernel(x, Wq, Wk, Wv, Wp):
    """Full-input / full-output causal MHA. x: (4, 2048, 1024) fp32;
    Wq/Wk/Wv/Wp: (1024, 1024) fp32. Returns (4, 2048, 1024) fp32."""
    global _COMPILED
    x = np.asarray(x, dtype=np.float32)
    Wq = np.asarray(Wq, dtype=np.float32)
    Wk = np.asarray(Wk, dtype=np.float32)
    Wv = np.asarray(Wv, dtype=np.float32)
    Wp = np.asarray(Wp, dtype=np.float32)
    assert x.shape == (B, T, D), x.shape

    if _COMPILED is None:
        _COMPILED = _Compiled(8)
    in_maps = [_make_core_inputs(x, Wq, Wk, Wv, Wp, c) for c in range(8)]
    results = _COMPILED.run(in_maps)

    out = np.empty((B, T, D), np.float32)
    for b in range(B):
        acc = (results[2 * b]["outt"].astype(np.float32)
               + results[2 * b + 1]["outt"].astype(np.float32))
        out[b] = acc.T
    return out
